# revision 1
# baseline (speedup 1.0000x reference)
"""DeepSeekMoE Trainium2 kernel (8 NeuronCores, data-parallel over tokens).

Reference computation (B=128, FEW=64, D=512, E=16, O=512, H=64, K=3):
  t = x.reshape(T=8192, D)
  gates = softmax(relu(t@gW1+gb1)@gW2+gb2)            # [T, E]
  h  = relu(einsum('td,edh->teh', t, W1) + b1)        # [T, E, H]
  eo = einsum('teh,eho->teo', h, W2) + b2             # [T, E, O]
  topv, topi = top_k(gates, 3); out_t = sum_k topv * eo[topi]
  out = mean over FEW  -> [B, 1, 1, O]

Kernel restructure (per core, 1024 tokens = 16 B-rows):
  gmask[t,e] = gates[t,e] if gates[t,e] in top-3 of row t else 0
  out_t      = sum_e gmask[t,e] * (relu(x_t W1_e + b1_e) W2_e) + gmask @ b2
and the FEW-mean is folded into W2/b2 (scaled by 1/64) with final
block-summing matmuls.

Precision: the expert path (MM1/MM2 and gate application) runs in fp16
with fp32 PSUM accumulation (~2.5e-4 rel error).  The gating network,
which must reproduce the reference's top-3 SELECTION exactly, runs the
hidden layer as a 3-pass fp16 hi/lo split (x = xh + xlo/2048,
gW1 = gh + gl/2048 -> xh@gh + (xh@gl + xlo@gh)/2048, accurate to ~1e-7)
and the tiny logits matmul in fp32, so no selection flips occur.

Experts are processed in pairs stacked along the 128-partition dim
(h2 = s*64 + h, e = 2*pair + s).  The per-pair gate broadcast across the
64 h-rows is a tiny constant "mask matmul" on the PE; gb2 is folded into
the logits matmul via a constant-1 row of the activation tile.
"""

import sys

import numpy as np

for _p in ("/opt/trn_rl_repo",):
    if _p not in sys.path:
        sys.path.insert(0, _p)

B, FEW, D = 128, 64, 512
E, O, H, TOPK = 16, 512, 64, 3
T = B * FEW            # 8192 tokens
NCORES = 8
TLOC = T // NCORES     # 1024 tokens per core
DT = 512               # tokens per quad tile
NDT = TLOC // DT       # 2 quad tiles per core
PAIRS = E // 2         # 8 expert pairs
NSLICE = TLOC // 128   # 8 token slices of 128 per core

_CACHE = {}


def _build_nc():
    import concourse.mybir as mybir
    import concourse.tile as tile
    from concourse import bacc

    f32 = mybir.dt.float32
    f32r = mybir.dt.float32r
    f16 = mybir.dt.float16
    AF = mybir.ActivationFunctionType
    ALU = mybir.AluOpType
    AX = mybir.AxisListType

    nc = bacc.Bacc("TRN2", target_bir_lowering=False, debug=False,
                   num_devices=NCORES)

    # ---- DRAM I/O ----------------------------------------------------------
    xt16_d = nc.dram_tensor("xt16", [D, TLOC], f16, kind="ExternalInput")
    xlo_d = nc.dram_tensor("xlo", [D, TLOC], f16, kind="ExternalInput")
    w1_d = nc.dram_tensor("w1", [128, 4, PAIRS, 128], f16, kind="ExternalInput")
    w2_d = nc.dram_tensor("w2", [128, PAIRS, O], f16, kind="ExternalInput")
    b1_d = nc.dram_tensor("b1", [128, PAIRS], f32, kind="ExternalInput")
    b2_d = nc.dram_tensor("b2", [E, O], f16, kind="ExternalInput")
    gw1_d = nc.dram_tensor("gw1", [128, 4, 2, H], f16, kind="ExternalInput")
    gb1_d = nc.dram_tensor("gb1", [H, 1], f32, kind="ExternalInput")
    gw2a_d = nc.dram_tensor("gw2a", [H + 1, E], f32, kind="ExternalInput")
    ident_d = nc.dram_tensor("ident", [128, 128], f32, kind="ExternalInput")
    maskp_d = nc.dram_tensor("maskp", [E, PAIRS, 128], f16, kind="ExternalInput")
    meanm_d = nc.dram_tensor("meanm", [128, NSLICE, E], f16, kind="ExternalInput")
    out_d = nc.dram_tensor("out", [16, O], f32, kind="ExternalOutput")

    with tile.TileContext(nc) as tc:
        with (
            tc.tile_pool(name="consts", bufs=1) as consts,
            tc.tile_pool(name="work", bufs=3) as work,
            tc.tile_pool(name="psH", bufs=2, space="PSUM") as psH,
            tc.tile_pool(name="psOut", bufs=2, space="PSUM") as psOut,
            tc.tile_pool(name="psSmall", bufs=2, space="PSUM") as psSmall,
            tc.tile_pool(name="psGbc", bufs=2, space="PSUM") as psGbc,
        ):
            # ---- resident SBUF loads --------------------------------------
            # Two HWDGE rings (SP + ACT); gating-critical tensors first on
            # each so the first matmuls start early.  xt/w1 split per d-chunk.
            xt16sb = consts.tile([128, 4, TLOC], f16)
            xlosb = consts.tile([128, 4, TLOC], f16)
            xlo_r = xlo_d.ap().rearrange("(j p) t -> p j t", p=128)
            gw1sb = consts.tile([128, 4, 2, H], f16)
            w1sb = consts.tile([128, 4, PAIRS, 128], f16)
            xt16_r = xt16_d.ap().rearrange("(j p) t -> p j t", p=128)
            gb1sb = consts.tile([H, 1], f32)
            b1sb = consts.tile([128, PAIRS], f32)
            # fp16 expert inputs first (MM1 work starts earliest), gating fp32
            # x after, weights/consts interleaved by first-use time.
            nc.sync.dma_start(out=gw1sb, in_=gw1_d.ap())
            nc.sync.dma_start(out=xt16sb[:, 0, :], in_=xt16_r[:, 0, :])
            nc.sync.dma_start(out=w1sb[:, 0, :, :], in_=w1_d.ap()[:, 0, :, :])
            nc.sync.dma_start(out=gb1sb, in_=gb1_d.ap())
            nc.sync.dma_start(out=b1sb, in_=b1_d.ap())
            nc.sync.dma_start(out=xlosb[:, 0, :], in_=xlo_r[:, 0, :])
            for j in range(1, 4):
                nc.sync.dma_start(out=xt16sb[:, j, :], in_=xt16_r[:, j, :])
                nc.sync.dma_start(out=w1sb[:, j, :, :], in_=w1_d.ap()[:, j, :, :])
                nc.sync.dma_start(out=xlosb[:, j, :], in_=xlo_r[:, j, :])
            identsb = consts.tile([128, 128], f32)
            nc.sync.dma_start(out=identsb, in_=ident_d.ap())
            maskpsb = consts.tile([E, PAIRS, 128], f16)
            nc.sync.dma_start(out=maskpsb, in_=maskp_d.ap())
            gw2asb = consts.tile([H + 1, E], f32)
            nc.sync.dma_start(out=gw2asb, in_=gw2a_d.ap())
            b2sb = consts.tile([E, O], f16)
            nc.sync.dma_start(out=b2sb, in_=b2_d.ap())
            w2sb = consts.tile([128, PAIRS, O], f16)
            for pr in range(PAIRS):
                nc.sync.dma_start(out=w2sb[:, pr, :], in_=w2_d.ap()[:, pr, :])
            meanmsb = consts.tile([128, NSLICE, E], f16)
            nc.sync.dma_start(out=meanmsb, in_=meanm_d.ap())


            # gating hidden activations for the whole core, row H is the
            # constant 1.0 row that folds gb2 into the logits matmul
            asb = consts.tile([H + 1, TLOC], f32)
            nc.vector.memset(asb[H:H + 1, :], 1.0)

            def mm1_pair(pair, t0):
                """MM1 + relu for one expert pair; returns hr tile."""
                psh = psH.tile([128, DT], f32, tag="psh",
                               name=f"psh_{t0}_{pair}")
                for j in range(4):
                    nc.tensor.matmul(psh, w1sb[:, j, pair, :],
                                     xt16sb[:, j, t0:t0 + DT],
                                     start=(j == 0), stop=(j == 3))
                hr = work.tile([128, DT], f16, tag="hr", bufs=9,
                               name=f"hr_{t0}_{pair}")
                nc.scalar.activation(hr, psh, AF.Relu,
                                     bias=b1sb[:, pair:pair + 1], scale=1.0)
                return hr

            for dti in range(NDT):
                t0 = dti * DT
                # ---- gating hidden: A^T = relu(gW1^T X^T + gb1) -----------
                # 3-pass fp16 hi/lo decomposition, exact to ~1e-7:
                #   A = xh@gh + (xh@gl + xl@gh) / 2048
                psA = psSmall.tile([H, DT], f32, tag="small")
                psA2 = psSmall.tile([H, DT], f32, tag="small")
                for j in range(4):
                    nc.tensor.matmul(psA, gw1sb[:, j, 0, :],
                                     xt16sb[:, j, t0:t0 + DT],
                                     start=(j == 0), stop=(j == 3))
                    nc.tensor.matmul(psA2, gw1sb[:, j, 1, :],
                                     xt16sb[:, j, t0:t0 + DT],
                                     start=(j == 0), stop=False)
                    nc.tensor.matmul(psA2, gw1sb[:, j, 0, :],
                                     xlosb[:, j, t0:t0 + DT],
                                     start=False, stop=(j == 3))
                sbA2 = work.tile([H, DT], f32, tag="sbA2")
                nc.vector.tensor_copy(sbA2, psA2)
                zsb = work.tile([H, DT], f32, tag="zsb")
                nc.vector.scalar_tensor_tensor(zsb, sbA2, 1.0 / 2048.0, psA,
                                               op0=ALU.mult, op1=ALU.add)
                nc.scalar.activation(asb[0:H, t0:t0 + DT], zsb, AF.Relu,
                                     bias=gb1sb, scale=1.0)
                # ---- logits + softmax + top-3 mask + transpose ------------
                # two expert MM1 pairs interleave with each slice's softmax
                # chain so the in-order PE never waits on DVE/ACT
                gmt = work.tile([E, DT], f16, tag="gmt")
                hrs = []
                for s in range(DT // 128):
                    hrs += [mm1_pair(p, t0) for p in (2 * s, 2 * s + 1)]
                    st = s * 128
                    psL = psSmall.tile([128, E], f32, tag="small")
                    nc.tensor.matmul(psL, asb[:, t0 + st:t0 + st + 128], gw2asb)
                    negmax = work.tile([128, 1], f32, tag="negmax")
                    nc.vector.tensor_reduce(negmax, psL, axis=AX.X, op=ALU.max,
                                            negate=True)
                    expd = work.tile([128, E], f32, tag="expd")
                    sume = work.tile([128, 1], f32, tag="sume")
                    nc.scalar.activation(expd, psL, AF.Exp, bias=negmax,
                                         scale=1.0, accum_out=sume)
                    rsum = work.tile([128, 1], f32, tag="rsum")
                    nc.vector.reciprocal(rsum, sume)
                    gfull = work.tile([128, E], f32, tag="gfull")
                    nc.vector.tensor_scalar_mul(gfull, expd, rsum)
                    top8 = work.tile([128, 8], f32, tag="top8")
                    nc.vector.max(top8, gfull)
                    gmask = work.tile([128, E], f32, tag="gmask")
                    nc.vector.scalar_tensor_tensor(gmask, gfull, top8[:, 2:3],
                                                   gfull, op0=ALU.is_ge,
                                                   op1=ALU.mult)
                    psGT = psSmall.tile([E, 128], f32, tag="small")
                    nc.tensor.transpose(psGT, gmask, identsb)
                    nc.scalar.copy(gmt[:, st:st + 128], psGT)

                # ---- gate-scale, then MM2 in two half-phases --------------
                hgs = []
                for pair in range(PAIRS):
                    psG = psGbc.tile([128, DT], f32, tag="psG",
                                     name=f"psG{dti}_{pair}")
                    nc.tensor.matmul(psG, maskpsb[:, pair, :], gmt)
                    hg = work.tile([128, DT], f16, tag="hg", bufs=9,
                                   name=f"hg{dti}_{pair}")
                    nc.vector.tensor_mul(hg, hrs[pair], psG)
                    hgs.append(hg)
                for half in range(2):
                    psO = [psOut.tile([128, O], f32, tag="psO",
                                      name=f"psO{dti}_{half}_{s}")
                           for s in range(2)]
                    outs = []
                    for pair in range(PAIRS):
                        for s in range(2):
                            st = (half * 2 + s) * 128
                            nc.tensor.matmul(psO[s], hgs[pair][:, st:st + 128],
                                             w2sb[:, pair, :],
                                             start=(pair == 0), stop=False)
                    for s in range(2):
                        st = (half * 2 + s) * 128
                        nc.tensor.matmul(psO[s], gmt[:, st:st + 128], b2sb,
                                         start=False, stop=True)
                    for s in range(2):
                        outsb = work.tile([128, O], f16, tag="outsb",
                                          name=f"outsb{dti}_{half}_{s}")
                        if s == 0:
                            nc.vector.tensor_copy(outsb, psO[s])
                        else:
                            nc.scalar.copy(outsb, psO[s])
                        outs.append(outsb)
                    psM = psGbc.tile([16, O], f32, tag="psG",
                                     name=f"psM{dti}_{half}")
                    for s in range(2):
                        s8 = dti * 4 + half * 2 + s
                        nc.tensor.matmul(psM, meanmsb[:, s8, :], outs[s],
                                         start=(s == 0), stop=(s == 1))
                    r0 = 8 * dti + 4 * half
                    rowsb = work.tile([16, O], f32, tag="rowsb",
                                      name=f"rowsb{dti}_{half}")
                    nc.vector.tensor_copy(rowsb, psM)
                    nc.sync.dma_start(out=out_d.ap()[r0:r0 + 4, :],
                                      in_=rowsb[r0:r0 + 4, :])


    nc.compile()
    return nc


def _host_inputs(x, gW1, gb1, gW2, gb2, W1, b1, W2, b2):
    """Per-core in_maps with all host-side layout transforms."""
    f = np.float32
    xt_full = np.ascontiguousarray(x.reshape(T, D).T.astype(f))       # [D, T]
    # W1 [E,D,H] -> [p, j, pair, s*64+h], e = 2*pair+s, d = 128*j+p
    w1sb = np.ascontiguousarray(
        W1.reshape(PAIRS, 2, 4, 128, H).transpose(3, 2, 0, 1, 4)
        .reshape(128, 4, PAIRS, 128).astype(f))
    # W2 [E,H,O] -> [s*64+h, pair, o], mean folded
    w2sb = np.ascontiguousarray(
        W2.reshape(PAIRS, 2, H, O).transpose(1, 2, 0, 3)
        .reshape(128, PAIRS, O).astype(f) / np.float32(FEW))
    b1sb = np.ascontiguousarray(
        b1.reshape(PAIRS, 2, H).transpose(1, 2, 0).reshape(128, PAIRS).astype(f))
    b2sb = np.ascontiguousarray(b2.astype(f) / np.float32(FEW))
    gw1f = gW1.reshape(4, 128, H).transpose(1, 0, 2).astype(f)  # [128,4,64]
    gw1hi = gw1f.astype(np.float16)
    gw1lo = ((gw1f - gw1hi.astype(f)) * 2048.0).astype(np.float16)
    gw1a = np.ascontiguousarray(
        np.stack([gw1hi, gw1lo], axis=2))                        # [128,4,2,64]
    gb1sb = np.ascontiguousarray(gb1.reshape(H, 1).astype(f))
    gw2a = np.ascontiguousarray(
        np.vstack([gW2.astype(f), gb2.reshape(1, E).astype(f)]))
    ident = np.eye(128, dtype=f)
    # maskp[e, pair, m] = 1 if e == 2*pair + m//64
    m = np.arange(128)
    pr = np.arange(PAIRS)
    ee = np.arange(E)
    maskp = (ee[:, None, None] == (2 * pr[None, :, None] + m[None, None, :] // 64)
             ).astype(f)
    # meanm[p, s, rrow] = 1 if rrow == 2*s + p//64
    ss = np.arange(NSLICE)
    rr = np.arange(E)
    meanm = (rr[None, None, :] == (2 * ss[None, :, None] + m[:, None, None] // 64)
             ).astype(f)

    h = np.float16
    shared = dict(w1=w1sb.astype(h), w2=w2sb.astype(h), b1=b1sb,
                  b2=b2sb.astype(h), gw1=gw1a, gb1=gb1sb, gw2a=gw2a,
                  ident=ident, maskp=maskp.astype(h), meanm=meanm.astype(h))
    in_maps = []
    for c in range(NCORES):
        im = dict(shared)
        xt_c = np.ascontiguousarray(xt_full[:, c * TLOC:(c + 1) * TLOC])
        xhi = xt_c.astype(h)
        im["xt16"] = xhi
        im["xlo"] = ((xt_c - xhi.astype(f)) * 2048.0).astype(h)
        in_maps.append(im)
    return in_maps


def kernel(x, gW1, gb1, gW2, gb2, W1, b1, W2, b2, _trace=False):
    from concourse.bass_utils import run_bass_kernel_spmd

    if "nc" not in _CACHE:
        _CACHE["nc"] = _build_nc()
    nc = _CACHE["nc"]
    args = [np.asarray(a, dtype=np.float32)
            for a in (x, gW1, gb1, gW2, gb2, W1, b1, W2, b2)]
    in_maps = _host_inputs(*args)
    try:
        kres = run_bass_kernel_spmd(nc, in_maps, core_ids=list(range(NCORES)),
                                    trace=_trace)
    except ModuleNotFoundError:
        # NTFF profile hook absent in this container; run without trace
        kres = run_bass_kernel_spmd(nc, in_maps, core_ids=list(range(NCORES)),
                                    trace=False)
    _CACHE["last_result"] = kres
    out = np.concatenate([kres.results[c]["out"] for c in range(NCORES)], axis=0)
    return out.reshape(B, 1, 1, O).astype(np.float32)



# revision 2
# speedup vs baseline: 1.2071x; 1.2071x over previous
"""DeepSeekMoE Trainium2 kernel v2 (8 NeuronCores, data-parallel over tokens).

Reference computation (B=128, FEW=64, D=512, E=16, O=512, H=64, K=3):
  t = x.reshape(T=8192, D)
  gates = softmax(relu(t@gW1+gb1)@gW2+gb2)            # [T, E]
  h  = relu(einsum('td,edh->teh', t, W1) + b1)        # [T, E, H]
  eo = einsum('teh,eho->teo', h, W2) + b2             # [T, E, O]
  topv, topi = top_k(gates, 3); out_t = sum_k topv * eo[topi]
  out = mean over FEW  -> [B, 1, 1, O]

v2 restructure (per core, 1024 tokens = 16 B-rows of 64):
  The FEW-mean makes the output a per-row SUM over tokens, so the
  per-token MM2 ([T,E,H]@[E,H,O], 32k PE cycles/core) collapses to a
  gate-weighted per-row reduction in H space:
     hbar[sh, e, r] = sum_{t in row r} gmask[t, e] * h[t, e-part, sh]
     out[o, r] = sum_e w2[sh, e, o]^T hbar + b2^T gsum
  MM1 emits h TOKEN-MAJOR (psHT[t, (4 pairs)x(128 sh)]) with b1 folded
  in by a K=16 ones-row matmul; hbar is 64 [128t x 128sh]@[128t x 16]
  matmuls against grsel (gate x row-onehot built by broadcast DVE ops);
  MM2 contracts only [128sh x 8r] per o-chunk.  PSUM accumulation
  respects the 2KB zero-region rule: each bank hosts one group at a
  time, opened by a zeroing matmul and closed before any reader.
  Rows 0-7 belong to the first 512-token tile, so the dti0 tail
  overlaps dti1 MM1 compute.  Warm-up matmuls on scratch data ramp the
  PE p-state during the initial DMA window.

  Gating runs 1-pass fp16; top-3 selection flips vs the fp32 reference
  are ~6/8192 tokens with tiny |g3-g4|; host-validated ~6e-3 total
  error against the 2e-2 budget.
"""

import sys

import numpy as np

for _p in ("/opt/trn_rl_repo",):
    if _p not in sys.path:
        sys.path.insert(0, _p)

B, FEW, D = 128, 64, 512
E, O, H, TOPK = 16, 512, 64, 3
T = B * FEW            # 8192 tokens
NCORES = 8
TLOC = T // NCORES     # 1024 tokens per core
DT = 512               # tokens per dti tile
NDT = TLOC // DT       # 2
PAIRS = E // 2         # 8 expert pairs
NCH = TLOC // 128      # 8 token chunks of 128
ROWS = TLOC // FEW     # 16 B-rows per core
RL = ROWS // NDT       # 8 rows per dti
N_WARM = 10            # PE warm-up matmuls

_CACHE = {}


def _build_nc():
    import concourse.mybir as mybir
    import concourse.tile as tile
    from concourse import bacc

    f32 = mybir.dt.float32
    f16 = mybir.dt.float16
    AF = mybir.ActivationFunctionType
    ALU = mybir.AluOpType
    AX = mybir.AxisListType

    nc = bacc.Bacc("TRN2", target_bir_lowering=False, debug=False,
                   num_devices=NCORES)

    # ---- DRAM I/O ----------------------------------------------------------
    xt16_d = nc.dram_tensor("xt16", [128, TLOC, 4], f16, kind="ExternalInput")
    gw1_d = nc.dram_tensor("gw1", [128, 4, H], f16, kind="ExternalInput")
    gb1_d = nc.dram_tensor("gb1", [H, 1], f32, kind="ExternalInput")
    gw2a_d = nc.dram_tensor("gw2a", [H + 1, E], f16, kind="ExternalInput")
    # w1 [d-part, half, j, (4 pairs x 128 sh)]
    w1_d = nc.dram_tensor("w1", [128, 2, 4, 512], f16, kind="ExternalInput")
    # b1r [16, half, 512]: row 0 = b1, rows 1..15 = 0
    b1r_d = nc.dram_tensor("b1r", [16, 2, 512], f16, kind="ExternalInput")
    w2_d = nc.dram_tensor("w2", [128, PAIRS, O], f16, kind="ExternalInput")
    b2_d = nc.dram_tensor("b2", [E, O], f16, kind="ExternalInput")
    # rowohc [p, ch, rl] = (rl == 2*ch + p//64), rl local to the dti
    rowohc_d = nc.dram_tensor("rowohc", [128, 4, RL], f16,
                              kind="ExternalInput")
    out_d = nc.dram_tensor("out", [128, 4, ROWS], f32, kind="ExternalOutput")

    with tile.TileContext(nc) as tc:
        with (
            tc.tile_pool(name="consts", bufs=1) as consts,
            tc.tile_pool(name="work", bufs=3) as work,
            tc.tile_pool(name="psHT", bufs=3, space="PSUM") as psHT,
            tc.tile_pool(name="psA", bufs=1, space="PSUM") as psAp,
            tc.tile_pool(name="psSmall", bufs=1, space="PSUM") as psSmall,
            tc.tile_pool(name="psB", bufs=1, space="PSUM") as psBp,
            tc.tile_pool(name="psTail", bufs=1, space="PSUM") as psTail,
        ):
            # ---- SBUF consts + DMA loads ---------------------------------
            gw1sb = consts.tile([128, 4, H], f16)
            xta = consts.tile([128, 128, 4], f16)
            xtb = consts.tile([128, DT - 128, 4], f16)
            xtc = consts.tile([128, DT, 4], f16)

            def xt_chunk(chg):
                """(tile, token offset within tile) for 128-token chunk."""
                if chg == 0:
                    return xta, 0
                if chg < 4:
                    return xtb, (chg - 1) * 128
                return xtc, (chg - 4) * 128

            gb1sb = consts.tile([H, 1], f32)
            gw2asb = consts.tile([H + 1, E], f16)
            w1sb0 = consts.tile([128, 4, 512], f16)
            w1sb1 = consts.tile([128, 4, 512], f16)
            b1rsb = consts.tile([16, 2, 512], f16)
            w2sb = consts.tile([128, PAIRS, O], f16)
            b2sb = consts.tile([E, O], f16)
            rowohcsb = consts.tile([128, 4, RL], f16)

            xt_r = xt16_d.ap()
            # SP queue: big tensors, ordered by first use
            nc.sync.dma_start(out=w1sb0, in_=w1_d.ap()[:, 0, :, :])
            nc.sync.dma_start(out=xta, in_=xt_r[:, 0:128, :])
            nc.sync.dma_start(out=xtb, in_=xt_r[:, 128:DT, :])
            nc.sync.dma_start(out=w1sb1, in_=w1_d.ap()[:, 1, :, :])
            nc.sync.dma_start(out=b1rsb, in_=b1r_d.ap())
            nc.sync.dma_start(out=xtc, in_=xt_r[:, DT:TLOC, :])
            nc.sync.dma_start(out=w2sb, in_=w2_d.ap())
            # Pool (SWDGE) queue: small consts, bypassing shared HWDGE
            nc.gpsimd.dma_start(out=gw1sb, in_=gw1_d.ap())
            nc.gpsimd.dma_start(out=gb1sb, in_=gb1_d.ap())
            nc.gpsimd.dma_start(out=gw2asb, in_=gw2a_d.ap())
            nc.gpsimd.dma_start(out=rowohcsb, in_=rowohc_d.ap())
            nc.gpsimd.dma_start(out=b2sb, in_=b2_d.ap())

            # device consts on DVE (idle early)
            scr16 = consts.tile([16, 128], f16)
            nc.vector.memset(scr16, 0.125)
            scrw = consts.tile([16, DT], f16)
            nc.vector.memset(scrw, 0.125)
            zerosW = consts.tile([16, DT], f16)
            nc.vector.memset(zerosW, 0.0)
            asb = consts.tile([H + 1, TLOC], f16)
            ones16 = consts.tile([16, 128], f16)
            nc.vector.memset(ones16, 1.0)
            nc.vector.memset(asb[H:H + 1, :], 1.0)

            gm = consts.tile([128, NCH, E], f16)        # per-chunk gate masks
            grsel2s = [None] * NDT
            hgbar16 = consts.tile([128, PAIRS, NDT, RL], f16)
            gsumAcc = consts.tile([E, NDT, RL], f32)
            gsum16 = consts.tile([E, NDT, RL], f16)
            outsb = consts.tile([128, 4, ROWS], f32)

            # psB banks: [128, 4, PAIRS, 2, RL] = full 2KB bank; hbar uses
            # the [:, 0] quarter; warm-up matmuls reuse psB0's whole bank.
            psB0 = psBp.tile([128, 4, PAIRS, 2, RL], f32, tag="psB0",
                             name="psB0")
            psB1 = psBp.tile([128, 4, PAIRS, 2, RL], f32, tag="psB1",
                             name="psB1")
            psBs = [psB0[:, 0, :, :, :], psB1[:, 0, :, :, :]]
            psAs = [None] * NDT
            # psL bank: logits region + per-chunk gsum region
            psLone = psSmall.tile([128, E], f32, tag="psL", name="psLone")
            # psOT: current dti's MM2 rows only
            psOT = psTail.tile([128, 4, RL], f32, tag="psOT", name="psOT")

            def warm(n):
                for wi in range(n):
                    nc.tensor.matmul(psB0[:, :, :, :, :], scr16, scrw)

            def bank_open(view, width):
                """Open an accumulation group, zeroing the whole bank."""
                nc.tensor.matmul(view, scr16[:, 0:view.partition_size()],
                                 zerosW[:, 0:width], start=True, stop=False)

            def gating(dti):
                """hidden: one accumulation group + one relu -> asb."""
                psA = psAp.tile([H, DT], f32, tag="psA", name=f"psA{dti}")
                psAs[dti] = psA
                bank_open(psA, DT)
                for c in range(4):
                    xt_t, toff = xt_chunk(dti * 4 + c)
                    for j in range(4):
                        nc.tensor.matmul(psA[:, c * 128:(c + 1) * 128],
                                         gw1sb[:, j, :],
                                         xt_t[:, toff:toff + 128, j],
                                         start=False,
                                         stop=(c == 3 and j == 3))
                t0 = dti * DT
                nc.scalar.activation(asb[0:H, t0:t0 + DT], psA, AF.Relu,
                                     bias=gb1sb, scale=1.0)

            def gating_slice(dti, s):
                chg = dti * 4 + s
                st = dti * DT + s * 128
                nc.tensor.matmul(psLone, asb[:, st:st + 128], gw2asb)
                negmax = work.tile([128, 1], f32, tag="negmax")
                nc.vector.tensor_reduce(negmax, psLone, axis=AX.X, op=ALU.max,
                                        negate=True)
                expd = work.tile([128, E], f32, tag="expd")
                sume = work.tile([128, 1], f32, tag="sume")
                nc.scalar.activation(expd, psLone, AF.Exp, bias=negmax,
                                     scale=1.0, accum_out=sume)
                rsum = work.tile([128, 1], f32, tag="rsum")
                nc.vector.reciprocal(rsum, sume)
                gfull = work.tile([128, E], f32, tag="gfull")
                nc.vector.tensor_scalar_mul(gfull, expd, rsum)
                top8 = work.tile([128, 8], f32, tag="top8")
                nc.vector.max(top8, gfull)
                nc.vector.scalar_tensor_tensor(gm[:, chg, :], gfull,
                                               top8[:, 2:3], gfull,
                                               op0=ALU.is_ge, op1=ALU.mult)
                # per-chunk gsum into the psL bank, then DVE-accumulate
                nc.tensor.matmul(psLone[0:E, 0:RL], gm[:, chg, :],
                                 rowohcsb[:, s, :])
                if s == 0:
                    nc.vector.tensor_copy(gsumAcc[:, dti, :], psLone[0:E, 0:RL])
                else:
                    nc.vector.tensor_tensor(out=gsumAcc[:, dti, :],
                                            in0=gsumAcc[:, dti, :],
                                            in1=psLone[0:E, 0:RL], op=ALU.add)

            def grsel_build(dti):
                """grsel2[p,ch,pr,s,rl] = gm[p,chg,2pr+s]*rowohc[p,ch,rl]."""
                g2 = work.tile([128, 4, PAIRS, 2, RL], f16, tag="grsel2",
                               bufs=2, name=f"grsel2_{dti}")
                for ch in range(4):
                    chg = dti * 4 + ch
                    in0 = (gm[:, chg, :]
                           .rearrange("p (pr s) -> p pr s", pr=PAIRS)
                           .unsqueeze(-1).broadcast_to([128, PAIRS, 2, RL]))
                    in1 = (rowohcsb[:, ch, :]
                           .unsqueeze(1).unsqueeze(1)
                           .broadcast_to([128, PAIRS, 2, RL]))
                    nc.vector.tensor_tensor(out=g2[:, ch, :, :, :],
                                            in0=in0, in1=in1, op=ALU.mult)
                grsel2s[dti] = g2

            def mm1(dti, ch, half):
                """4 j-matmuls + K=16 bias matmul -> psHT; ACT relu -> hrT."""
                ps = psHT.tile([128, 512], f32, tag="psHT",
                               name=f"psHT{dti}_{ch}_{half}")
                xt_t, toff = xt_chunk(dti * 4 + ch)
                w1t = w1sb0 if half == 0 else w1sb1
                for j in range(4):
                    nc.tensor.matmul(ps, xt_t[:, toff:toff + 128, j],
                                     w1t[:, j, :],
                                     start=(j == 0), stop=False)
                nc.tensor.matmul(ps, ones16, b1rsb[:, half, :],
                                 start=False, stop=True)
                hrT = work.tile([128, 4, 128], f16, tag="hrT", bufs=8,
                                name=f"hrT{dti}_{ch}_{half}")
                nc.scalar.activation(hrT[:, :, :], ps, AF.Relu, scale=1.0)
                return hrT

            def hbar_mms(dti, ch, half, hrT):
                """psB_d[:, pr, s, :] += hrT[:, q, :]^T @ grsel2[:, ch, pr]."""
                g2 = grsel2s[dti]
                for q in range(4):
                    pr = half * 4 + q
                    nc.tensor.matmul(psBs[dti][:, pr, :, :], hrT[:, q, :],
                                     g2[:, ch, pr, :, :],
                                     start=False,
                                     stop=(ch == 3 and half == 1 and q == 3))

            def tail(dti):
                """converts; MM2 for this dti's rows; store + DMA out."""
                psB = psBs[dti]
                nc.vector.tensor_copy(gsum16[:, dti, :], gsumAcc[:, dti, :])
                nc.vector.tensor_copy(hgbar16[0:64, :, dti, :],
                                      psB[0:64, :, 0, :])
                nc.vector.tensor_copy(hgbar16[64:128, :, dti, :],
                                      psB[64:128, :, 1, :])
                bank_open(psOT, 4 * RL)
                r0 = dti * RL
                for oc in range(4):
                    o0 = oc * 128
                    nc.tensor.matmul(psOT[:, oc, :],
                                     b2sb[:, o0:o0 + 128], gsum16[:, dti, :],
                                     start=False, stop=False)
                    for pair in range(PAIRS):
                        nc.tensor.matmul(psOT[:, oc, :],
                                         w2sb[:, pair, o0:o0 + 128],
                                         hgbar16[:, pair, dti, :],
                                         start=False,
                                         stop=(oc == 3 and pair == PAIRS - 1))
                for och in range(2):
                    oc0 = 2 * och
                    if och == 0:
                        nc.vector.tensor_copy(outsb[:, oc0:oc0 + 2, r0:r0 + RL],
                                              psOT[:, oc0:oc0 + 2, :])
                    else:
                        nc.scalar.copy(outsb[:, oc0:oc0 + 2, r0:r0 + RL],
                                       psOT[:, oc0:oc0 + 2, :])
                    eng = nc.scalar if och == 0 else nc.sync
                    eng.dma_start(out=out_d.ap()[:, oc0:oc0 + 2, r0:r0 + RL],
                                  in_=outsb[:, oc0:oc0 + 2, r0:r0 + RL])

            # ---- schedule ----------------------------------------------
            mm1_hrT = {}

            def MM1(d, c, h):
                mm1_hrT[(d, c, h)] = mm1(d, c, h)

            def HB(d, c, h):
                hbar_mms(d, c, h, mm1_hrT[(d, c, h)])

            warm(N_WARM)
            bank_open(psB0[:, :, :, :, :], DT)
            bank_open(psB1[:, :, :, :, :], DT)
            MM1(0, 0, 0)
            MM1(0, 1, 0)
            gating(0)
            MM1(0, 2, 0); gating_slice(0, 0)
            MM1(0, 3, 0); gating_slice(0, 1)
            MM1(0, 0, 1); gating_slice(0, 2)
            MM1(0, 1, 1); gating_slice(0, 3); grsel_build(0)
            MM1(0, 2, 1); HB(0, 0, 0); HB(0, 1, 0)
            MM1(0, 3, 1); HB(0, 2, 0); HB(0, 3, 0)
            MM1(1, 0, 0); HB(0, 0, 1); HB(0, 1, 1)
            MM1(1, 1, 0); HB(0, 2, 1); HB(0, 3, 1)
            gating(1)
            tail(0)
            MM1(1, 2, 0); gating_slice(1, 0)
            MM1(1, 3, 0); gating_slice(1, 1)
            MM1(1, 0, 1); gating_slice(1, 2)
            MM1(1, 1, 1); gating_slice(1, 3); grsel_build(1)
            MM1(1, 2, 1); HB(1, 0, 0); HB(1, 1, 0)
            MM1(1, 3, 1); HB(1, 2, 0); HB(1, 3, 0)
            HB(1, 0, 1); HB(1, 1, 1); HB(1, 2, 1); HB(1, 3, 1)
            tail(1)

    nc.compile()
    return nc


def _host_inputs(x, gW1, gb1, gW2, gb2, W1, b1, W2, b2):
    """Per-core in_maps with all host-side layout transforms."""
    f = np.float32
    h = np.float16
    # xt [p, t, j] = x[t, j*128+p]
    xt_full = np.ascontiguousarray(
        x.reshape(T, 4, 128).transpose(2, 0, 1).astype(f))   # [128, T, 4]
    # W1 [E,D,H]: e=8*half+2*q+s, d=128*j+p -> w1 [p, half, j, q*128+s*64+h]
    w1sb = np.ascontiguousarray(
        W1.reshape(2, 4, 2, 4, 128, H)            # (half, q, s, j, p, h)
        .transpose(4, 0, 3, 1, 2, 5)              # (p, half, j, q, s, h)
        .reshape(128, 2, 4, 512).astype(h))
    # b1r [16, half, 512]: row 0 = b1[(half,q,s), h], rows 1..15 = 0
    b1r = np.zeros((16, 2, 512), dtype=h)
    b1r[0] = b1.reshape(2, 4, 2, H).reshape(2, 512).astype(h)
    # W2 [E,H,O] -> [s*64+h, pair, o], mean folded; pair = 4*half+q, e=2*pair+s
    w2sb = np.ascontiguousarray(
        W2.reshape(PAIRS, 2, H, O).transpose(1, 2, 0, 3)
        .reshape(128, PAIRS, O).astype(f) / np.float32(FEW)).astype(h)
    b2sb = np.ascontiguousarray(b2.astype(f) / np.float32(FEW)).astype(h)
    gw1a = np.ascontiguousarray(
        gW1.reshape(4, 128, H).transpose(1, 0, 2).astype(h))   # [128,4,64]
    gb1sb = np.ascontiguousarray(gb1.reshape(H, 1).astype(f))
    gw2a = np.ascontiguousarray(
        np.vstack([gW2.astype(f), gb2.reshape(1, E).astype(f)])).astype(h)
    p = np.arange(128)
    ch = np.arange(4)
    rl = np.arange(RL)
    rowohc = (rl[None, None, :] ==
              (2 * ch[None, :, None] + p[:, None, None] // 64)).astype(h)

    shared = dict(w1=w1sb, b1r=b1r, w2=w2sb, b2=b2sb, gw1=gw1a, gb1=gb1sb,
                  gw2a=gw2a, rowohc=rowohc)
    in_maps = []
    for c in range(NCORES):
        im = dict(shared)
        im["xt16"] = np.ascontiguousarray(
            xt_full[:, c * TLOC:(c + 1) * TLOC, :]).astype(h)
        in_maps.append(im)
    return in_maps


def kernel(x, gW1, gb1, gW2, gb2, W1, b1, W2, b2, _trace=False):
    from concourse.bass_utils import run_bass_kernel_spmd

    if "nc" not in _CACHE:
        _CACHE["nc"] = _build_nc()
    nc = _CACHE["nc"]
    args = [np.asarray(a, dtype=np.float32)
            for a in (x, gW1, gb1, gW2, gb2, W1, b1, W2, b2)]
    in_maps = _host_inputs(*args)
    try:
        kres = run_bass_kernel_spmd(nc, in_maps, core_ids=list(range(NCORES)),
                                    trace=_trace)
    except ModuleNotFoundError:
        kres = run_bass_kernel_spmd(nc, in_maps, core_ids=list(range(NCORES)),
                                    trace=False)
    _CACHE["last_result"] = kres
    # out_d [128, 4, ROWS]: out[p, oc, r] -> full[b = c*16+r, o = oc*128+p]
    outs = []
    for c in range(NCORES):
        arr = kres.results[c]["out"]                  # [128, 4, 16]
        outs.append(np.transpose(arr, (2, 1, 0)).reshape(ROWS, O))
    out = np.concatenate(outs, axis=0)
    return out.reshape(B, 1, 1, O).astype(np.float32)


# revision 3
# speedup vs baseline: 1.2110x; 1.0032x over previous
"""DeepSeekMoE Trainium2 kernel v2 (8 NeuronCores, data-parallel over tokens).

Reference computation (B=128, FEW=64, D=512, E=16, O=512, H=64, K=3):
  t = x.reshape(T=8192, D)
  gates = softmax(relu(t@gW1+gb1)@gW2+gb2)            # [T, E]
  h  = relu(einsum('td,edh->teh', t, W1) + b1)        # [T, E, H]
  eo = einsum('teh,eho->teo', h, W2) + b2             # [T, E, O]
  topv, topi = top_k(gates, 3); out_t = sum_k topv * eo[topi]
  out = mean over FEW  -> [B, 1, 1, O]

v2 restructure (per core, 1024 tokens = 16 B-rows of 64):
  The FEW-mean makes the output a per-row SUM over tokens, so the
  per-token MM2 ([T,E,H]@[E,H,O], 32k PE cycles/core) collapses to a
  gate-weighted per-row reduction in H space:
     hbar[sh, e, r] = sum_{t in row r} gmask[t, e] * h[t, e-part, sh]
     out[o, r] = sum_e w2[sh, e, o]^T hbar + b2^T gsum
  MM1 emits h TOKEN-MAJOR (psHT[t, (4 pairs)x(128 sh)]) with b1 folded
  in by a K=16 ones-row matmul; hbar is 64 [128t x 128sh]@[128t x 16]
  matmuls against grsel (gate x row-onehot built by broadcast DVE ops);
  MM2 contracts only [128sh x 8r] per o-chunk.  PSUM accumulation
  respects the 2KB zero-region rule: each bank hosts one group at a
  time, opened by a zeroing matmul and closed before any reader.
  Rows 0-7 belong to the first 512-token tile, so the dti0 tail
  overlaps dti1 MM1 compute.  Warm-up matmuls on scratch data ramp the
  PE p-state during the initial DMA window.

  Gating runs 1-pass fp16; top-3 selection flips vs the fp32 reference
  are ~6/8192 tokens with tiny |g3-g4|; host-validated ~6e-3 total
  error against the 2e-2 budget.
"""

import sys

import numpy as np

for _p in ("/opt/trn_rl_repo",):
    if _p not in sys.path:
        sys.path.insert(0, _p)

B, FEW, D = 128, 64, 512
E, O, H, TOPK = 16, 512, 64, 3
T = B * FEW            # 8192 tokens
NCORES = 8
TLOC = T // NCORES     # 1024 tokens per core
DT = 512               # tokens per dti tile
NDT = TLOC // DT       # 2
PAIRS = E // 2         # 8 expert pairs
NCH = TLOC // 128      # 8 token chunks of 128
ROWS = TLOC // FEW     # 16 B-rows per core
RL = ROWS // NDT       # 8 rows per dti
N_WARM = 10            # PE warm-up matmuls

_CACHE = {}


def _build_nc():
    import concourse.mybir as mybir
    import concourse.tile as tile
    from concourse import bacc

    f32 = mybir.dt.float32
    f16 = mybir.dt.float16
    DR = mybir.MatmulPerfMode.DoubleRow
    AF = mybir.ActivationFunctionType
    ALU = mybir.AluOpType
    AX = mybir.AxisListType

    nc = bacc.Bacc("TRN2", target_bir_lowering=False, debug=False,
                   num_devices=NCORES)

    # ---- DRAM I/O ----------------------------------------------------------
    f8 = mybir.dt.float8e4
    xh_d = nc.dram_tensor("xh8", [128, 2, 2, TLOC], f8, kind="ExternalInput")
    xl_d = nc.dram_tensor("xl8", [128, 2, 2, TLOC], f8, kind="ExternalInput")
    gw1_d = nc.dram_tensor("gw18", [128, 2, 4, H], f8, kind="ExternalInput")
    gb1_d = nc.dram_tensor("gb1", [H, 1], f32, kind="ExternalInput")
    gw2a_d = nc.dram_tensor("gw2a", [H + 1, E], f16, kind="ExternalInput")
    # w1 [d-part, half, hl, j, (4 pairs x 128 sh)]
    w1_d = nc.dram_tensor("w18", [128, 2, 2, 4, 512], f8, kind="ExternalInput")
    # b1r [16, half, 512]: row 0 = b1, rows 1..15 = 0
    b1r_d = nc.dram_tensor("b1r", [16, 2, 512], f16, kind="ExternalInput")
    w2_d = nc.dram_tensor("w2", [128, PAIRS, O], f16, kind="ExternalInput")
    b2_d = nc.dram_tensor("b2", [E, O], f16, kind="ExternalInput")
    # rowohc [p, ch, rl] = (rl == 2*ch + p//64), rl local to the dti
    rowohc_d = nc.dram_tensor("rowohc", [128, 4, RL], f16,
                              kind="ExternalInput")
    out_d = nc.dram_tensor("out", [128, 4, ROWS], f32, kind="ExternalOutput")

    with tile.TileContext(nc) as tc:
        with (
            tc.tile_pool(name="consts", bufs=1) as consts,
            tc.tile_pool(name="work", bufs=3) as work,
            tc.tile_pool(name="psHT", bufs=3, space="PSUM") as psHT,
            tc.tile_pool(name="psA", bufs=1, space="PSUM") as psAp,
            tc.tile_pool(name="psSmall", bufs=1, space="PSUM") as psSmall,
            tc.tile_pool(name="psB", bufs=1, space="PSUM") as psBp,
            tc.tile_pool(name="psTail", bufs=1, space="PSUM") as psTail,
        ):
            # ---- SBUF consts + DMA loads ---------------------------------
            f8 = mybir.dt.float8e4
            gw1h = consts.tile([128, 4, H], f8)
            gw1l = consts.tile([128, 4, H], f8)
            xts = {}
            for hl in range(2):
                xts[(hl, 0)] = consts.tile([128, 2, 2, 128], f8,
                                           name=f"xt{hl}a")
                xts[(hl, 1)] = consts.tile([128, 2, 2, DT - 128], f8,
                                           name=f"xt{hl}b")
                xts[(hl, 2)] = consts.tile([128, 2, 2, DT], f8,
                                           name=f"xt{hl}c")

            def xt_chunk(chg, hl):
                """(tile, token offset within tile) for 128-token chunk."""
                if chg == 0:
                    return xts[(hl, 0)], 0
                if chg < 4:
                    return xts[(hl, 1)], (chg - 1) * 128
                return xts[(hl, 2)], (chg - 4) * 128

            gb1sb = consts.tile([H, 1], f32)
            gw2asb = consts.tile([H + 1, E], f16)
            w1s = {(half, hl): consts.tile([128, 4, 512], f8,
                                           name=f"w1s{half}{hl}")
                   for half in range(2) for hl in range(2)}
            b1rsb = consts.tile([16, 2, 512], f16)
            w2sb = consts.tile([128, PAIRS, O], f16)
            b2sb = consts.tile([E, O], f16)
            rowohcsb = consts.tile([128, 4, RL], f16)

            xh_r = xh_d.ap()
            xl_r = xl_d.ap()
            # SP queue: big tensors, ordered by first use
            nc.sync.dma_start(out=w1s[(0, 0)], in_=w1_d.ap()[:, 0, 0, :, :])
            nc.sync.dma_start(out=xts[(0, 0)], in_=xh_r[:, :, :, 0:128])
            nc.sync.dma_start(out=xts[(0, 1)], in_=xh_r[:, :, :, 128:DT])
            nc.sync.dma_start(out=xts[(1, 0)], in_=xl_r[:, :, :, 0:128])
            nc.sync.dma_start(out=xts[(1, 1)], in_=xl_r[:, :, :, 128:DT])
            nc.sync.dma_start(out=w1s[(0, 1)], in_=w1_d.ap()[:, 0, 1, :, :])
            nc.sync.dma_start(out=b1rsb, in_=b1r_d.ap())
            nc.sync.dma_start(out=w1s[(1, 0)], in_=w1_d.ap()[:, 1, 0, :, :])
            nc.sync.dma_start(out=w1s[(1, 1)], in_=w1_d.ap()[:, 1, 1, :, :])
            nc.sync.dma_start(out=xts[(0, 2)], in_=xh_r[:, :, :, DT:TLOC])
            nc.sync.dma_start(out=xts[(1, 2)], in_=xl_r[:, :, :, DT:TLOC])
            nc.sync.dma_start(out=w2sb, in_=w2_d.ap())
            # Pool (SWDGE) queue: small consts, bypassing shared HWDGE
            nc.gpsimd.dma_start(out=gw1h, in_=gw1_d.ap()[:, 0, :, :])
            nc.gpsimd.dma_start(out=gw1l, in_=gw1_d.ap()[:, 1, :, :])
            nc.gpsimd.dma_start(out=gb1sb, in_=gb1_d.ap())
            nc.gpsimd.dma_start(out=gw2asb, in_=gw2a_d.ap())
            nc.gpsimd.dma_start(out=rowohcsb, in_=rowohc_d.ap())
            nc.gpsimd.dma_start(out=b2sb, in_=b2_d.ap())

            # device consts on DVE (idle early)
            scr16 = consts.tile([16, 128], f16)
            nc.vector.memset(scr16, 0.125)
            scrw = consts.tile([16, DT], f16)
            nc.vector.memset(scrw, 0.125)
            zerosW = consts.tile([16, DT], f16)
            nc.vector.memset(zerosW, 0.0)
            asb = consts.tile([H + 1, TLOC], f16)
            ones16 = consts.tile([16, 128], f16)
            nc.vector.memset(ones16, 1.0)
            nc.vector.memset(asb[H:H + 1, :], 1.0)

            gm = consts.tile([128, NCH, E], f16)        # per-chunk gate masks
            grsel2s = [None] * NDT
            hgbar16 = consts.tile([128, PAIRS, NDT, RL], f16)
            gsumAcc = consts.tile([E, NDT, RL], f32)
            gsum16 = consts.tile([E, NDT, RL], f16)
            outsb = consts.tile([128, 4, ROWS], f32)

            # psB banks: [128, 4, PAIRS, 2, RL] = full 2KB bank; hbar uses
            # the [:, 0] quarter; warm-up matmuls reuse psB0's whole bank.
            psB0 = psBp.tile([128, 4, PAIRS, 2, RL], f32, tag="psB0",
                             name="psB0")
            psB1 = psBp.tile([128, 4, PAIRS, 2, RL], f32, tag="psB1",
                             name="psB1")
            psBs = [psB0[:, 0, :, :, :], psB1[:, 0, :, :, :]]
            psAs = [None] * NDT
            # psL bank: logits region + per-chunk gsum region
            psLone = psSmall.tile([128, E], f32, tag="psL", name="psLone")
            # psOT: current dti's MM2 rows only
            psOT = psTail.tile([128, 4, RL], f32, tag="psOT", name="psOT")

            def warm(n):
                for wi in range(n):
                    nc.tensor.matmul(psB0[:, :, :, :, :], scr16, scrw)

            def bank_open(view, width):
                """Open an accumulation group, zeroing the whole bank."""
                nc.tensor.matmul(view, scr16[:, 0:view.partition_size()],
                                 zerosW[:, 0:width], start=True, stop=False)

            def gating(dti):
                """hidden: fp8 3-term DoubleRow group + one relu -> asb."""
                psA = psAp.tile([H, DT], f32, tag="psA", name=f"psA{dti}")
                psAs[dti] = psA
                bank_open(psA, DT)
                terms = [(gw1h, 0), (gw1h, 1), (gw1l, 0)]   # (gw, x-hl)
                for ti, (gw, xhl) in enumerate(terms):
                    for c in range(4):
                        xt_t, toff = xt_chunk(dti * 4 + c, xhl)
                        for jp in range(2):
                            nc.tensor.matmul(
                                psA[:, c * 128:(c + 1) * 128],
                                gw[:, 2 * jp:2 * jp + 2, :],
                                xt_t[:, jp, :, toff:toff + 128],
                                start=False,
                                stop=(ti == 2 and c == 3 and jp == 1),
                                perf_mode=DR)
                t0 = dti * DT
                nc.scalar.activation(asb[0:H, t0:t0 + DT], psA, AF.Relu,
                                     bias=gb1sb, scale=1.0 / 4096.0)

            def gating_slice(dti, s):
                chg = dti * 4 + s
                st = dti * DT + s * 128
                nc.tensor.matmul(psLone, asb[:, st:st + 128], gw2asb)
                negmax = work.tile([128, 1], f32, tag="negmax")
                nc.vector.tensor_reduce(negmax, psLone, axis=AX.X, op=ALU.max,
                                        negate=True)
                expd = work.tile([128, E], f32, tag="expd")
                sume = work.tile([128, 1], f32, tag="sume")
                nc.scalar.activation(expd, psLone, AF.Exp, bias=negmax,
                                     scale=1.0, accum_out=sume)
                rsum = work.tile([128, 1], f32, tag="rsum")
                nc.vector.reciprocal(rsum, sume)
                gfull = work.tile([128, E], f32, tag="gfull")
                nc.vector.tensor_scalar_mul(gfull, expd, rsum)
                top8 = work.tile([128, 8], f32, tag="top8")
                nc.vector.max(top8, gfull)
                nc.vector.scalar_tensor_tensor(gm[:, chg, :], gfull,
                                               top8[:, 2:3], gfull,
                                               op0=ALU.is_ge, op1=ALU.mult)
                # per-chunk gsum into the psL bank, then DVE-accumulate
                nc.tensor.matmul(psLone[0:E, 0:RL], gm[:, chg, :],
                                 rowohcsb[:, s, :])
                if s == 0:
                    nc.vector.tensor_copy(gsumAcc[:, dti, :], psLone[0:E, 0:RL])
                else:
                    nc.vector.tensor_tensor(out=gsumAcc[:, dti, :],
                                            in0=gsumAcc[:, dti, :],
                                            in1=psLone[0:E, 0:RL], op=ALU.add)

            def grsel_build(dti):
                """grsel2[p,ch,pr,s,rl] = gm[p,chg,2pr+s]*rowohc[p,ch,rl]."""
                g2 = work.tile([128, 4, PAIRS, 2, RL], f16, tag="grsel2",
                               bufs=2, name=f"grsel2_{dti}")
                for ch in range(4):
                    chg = dti * 4 + ch
                    in0 = (gm[:, chg, :]
                           .rearrange("p (pr s) -> p pr s", pr=PAIRS)
                           .unsqueeze(-1).broadcast_to([128, PAIRS, 2, RL]))
                    in1 = (rowohcsb[:, ch, :]
                           .unsqueeze(1).unsqueeze(1)
                           .broadcast_to([128, PAIRS, 2, RL]))
                    nc.vector.tensor_tensor(out=g2[:, ch, :, :, :],
                                            in0=in0, in1=in1, op=ALU.mult)
                grsel2s[dti] = g2

            def mm1(dti, ch, half):
                """fp8 3-term DoubleRow + K=16 bias matmul; ACT relu."""
                ps = psHT.tile([128, 512], f32, tag="psHT",
                               name=f"psHT{dti}_{ch}_{half}")
                terms = [(0, 0), (1, 0), (0, 1)]            # (x-hl, w-hl)
                first = True
                for xhl, whl in terms:
                    xt_t, toff = xt_chunk(dti * 4 + ch, xhl)
                    w1t = w1s[(half, whl)]
                    for jp in range(2):
                        nc.tensor.matmul(
                            ps,
                            xt_t[:, jp, :, toff:toff + 128],
                            w1t[:, 2 * jp:2 * jp + 2, :],
                            start=first, stop=False, perf_mode=DR)
                        first = False
                nc.tensor.matmul(ps, ones16, b1rsb[:, half, :],
                                 start=False, stop=True)
                hrT = work.tile([128, 4, 128], f16, tag="hrT", bufs=8,
                                name=f"hrT{dti}_{ch}_{half}")
                nc.scalar.activation(hrT[:, :, :], ps, AF.Relu,
                                     scale=1.0 / 4096.0)
                return hrT

            def hbar_mms(dti, ch, half, hrT):
                """psB_d[:, pr, s, :] += hrT[:, q, :]^T @ grsel2[:, ch, pr]."""
                g2 = grsel2s[dti]
                for q in range(4):
                    pr = half * 4 + q
                    nc.tensor.matmul(psBs[dti][:, pr, :, :], hrT[:, q, :],
                                     g2[:, ch, pr, :, :],
                                     start=False,
                                     stop=(ch == 3 and half == 1 and q == 3))

            def tail(dti):
                """converts; MM2 for this dti's rows; store + DMA out."""
                psB = psBs[dti]
                nc.vector.tensor_copy(gsum16[:, dti, :], gsumAcc[:, dti, :])
                nc.vector.tensor_copy(hgbar16[0:64, :, dti, :],
                                      psB[0:64, :, 0, :])
                nc.vector.tensor_copy(hgbar16[64:128, :, dti, :],
                                      psB[64:128, :, 1, :])
                bank_open(psOT, 4 * RL)
                r0 = dti * RL
                for oc in range(4):
                    o0 = oc * 128
                    nc.tensor.matmul(psOT[:, oc, :],
                                     b2sb[:, o0:o0 + 128], gsum16[:, dti, :],
                                     start=False, stop=False)
                    for pair in range(PAIRS):
                        nc.tensor.matmul(psOT[:, oc, :],
                                         w2sb[:, pair, o0:o0 + 128],
                                         hgbar16[:, pair, dti, :],
                                         start=False,
                                         stop=(oc == 3 and pair == PAIRS - 1))
                for och in range(2):
                    oc0 = 2 * och
                    if och == 0:
                        nc.vector.tensor_copy(outsb[:, oc0:oc0 + 2, r0:r0 + RL],
                                              psOT[:, oc0:oc0 + 2, :])
                    else:
                        nc.scalar.copy(outsb[:, oc0:oc0 + 2, r0:r0 + RL],
                                       psOT[:, oc0:oc0 + 2, :])
                    eng = nc.scalar if och == 0 else nc.sync
                    eng.dma_start(out=out_d.ap()[:, oc0:oc0 + 2, r0:r0 + RL],
                                  in_=outsb[:, oc0:oc0 + 2, r0:r0 + RL])

            # ---- schedule ----------------------------------------------
            mm1_hrT = {}

            def MM1(d, c, h):
                mm1_hrT[(d, c, h)] = mm1(d, c, h)

            def HB(d, c, h):
                hbar_mms(d, c, h, mm1_hrT[(d, c, h)])

            warm(N_WARM)
            bank_open(psB0[:, :, :, :, :], DT)
            bank_open(psB1[:, :, :, :, :], DT)
            MM1(0, 0, 0)
            MM1(0, 1, 0)
            gating(0)
            MM1(0, 2, 0); gating_slice(0, 0)
            MM1(0, 3, 0); gating_slice(0, 1)
            MM1(0, 0, 1); gating_slice(0, 2)
            MM1(0, 1, 1); gating_slice(0, 3); grsel_build(0)
            MM1(0, 2, 1); HB(0, 0, 0); HB(0, 1, 0)
            MM1(0, 3, 1); HB(0, 2, 0); HB(0, 3, 0)
            MM1(1, 0, 0); HB(0, 0, 1); HB(0, 1, 1)
            MM1(1, 1, 0); HB(0, 2, 1); HB(0, 3, 1)
            gating(1)
            tail(0)
            MM1(1, 2, 0); gating_slice(1, 0)
            MM1(1, 3, 0); gating_slice(1, 1)
            MM1(1, 0, 1); gating_slice(1, 2)
            MM1(1, 1, 1); gating_slice(1, 3); grsel_build(1)
            MM1(1, 2, 1); HB(1, 0, 0); HB(1, 1, 0)
            MM1(1, 3, 1); HB(1, 2, 0); HB(1, 3, 0)
            HB(1, 0, 1); HB(1, 1, 1); HB(1, 2, 1); HB(1, 3, 1)
            tail(1)

    nc.compile()
    return nc


def _host_inputs(x, gW1, gb1, gW2, gb2, W1, b1, W2, b2):
    """Per-core in_maps with all host-side layout transforms."""
    import ml_dtypes
    f = np.float32
    h = np.float16
    f8 = ml_dtypes.float8_e4m3fn
    # xt [p, jp, j2, t] = x[t, (2*jp+j2)*128+p]; fp8 hi/lo split
    xt_full = np.ascontiguousarray(
        x.reshape(T, 2, 2, 128).transpose(3, 1, 2, 0).astype(f))
    xt_s = xt_full * np.float32(16.0)
    xt_hi = xt_s.astype(f8)
    xt_lo = (xt_s - xt_hi.astype(f)).astype(f8)
    # W1 [E,D,H]: e=8*half+2*q+s, d=128*j+p -> w1 [p, half, hl, j, qsh]
    w1f = np.ascontiguousarray(
        W1.reshape(2, 4, 2, 4, 128, H)            # (half, q, s, j, p, h)
        .transpose(4, 0, 3, 1, 2, 5)              # (p, half, j, q, s, h)
        .reshape(128, 2, 4, 512).astype(f))
    w1f *= np.float32(256.0)
    w1hi = w1f.astype(f8)
    w1lo = (w1f - w1hi.astype(f)).astype(f8)
    w1sb = np.ascontiguousarray(
        np.stack([w1hi, w1lo], axis=2))           # [128, 2, 2, 4, 512]
    # b1r [16, half, 512]: row 0 = b1[(half,q,s), h], rows 1..15 = 0
    b1r = np.zeros((16, 2, 512), dtype=h)
    b1r[0] = (b1.reshape(2, 4, 2, H).reshape(2, 512)
              * np.float32(4096.0)).astype(h)
    # W2 [E,H,O] -> [s*64+h, pair, o], mean folded; pair = 4*half+q, e=2*pair+s
    w2sb = np.ascontiguousarray(
        W2.reshape(PAIRS, 2, H, O).transpose(1, 2, 0, 3)
        .reshape(128, PAIRS, O).astype(f) / np.float32(FEW)).astype(h)
    b2sb = np.ascontiguousarray(b2.astype(f) / np.float32(FEW)).astype(h)
    gw1f = gW1.reshape(4, 128, H).transpose(1, 0, 2).astype(f)   # [128,4,64]
    gw1f = gw1f * np.float32(256.0)
    gw1hi = gw1f.astype(f8)
    gw1lo = (gw1f - gw1hi.astype(f)).astype(f8)
    gw1a = np.ascontiguousarray(np.stack([gw1hi, gw1lo], axis=1))
    gb1sb = np.ascontiguousarray(gb1.reshape(H, 1).astype(f))
    gw2a = np.ascontiguousarray(
        np.vstack([gW2.astype(f), gb2.reshape(1, E).astype(f)])).astype(h)
    p = np.arange(128)
    ch = np.arange(4)
    rl = np.arange(RL)
    rowohc = (rl[None, None, :] ==
              (2 * ch[None, :, None] + p[:, None, None] // 64)).astype(h)

    shared = dict(w18=w1sb, b1r=b1r, w2=w2sb, b2=b2sb, gw18=gw1a,
                  gb1=gb1sb, gw2a=gw2a, rowohc=rowohc)
    in_maps = []
    for c in range(NCORES):
        im = dict(shared)
        im["xh8"] = np.ascontiguousarray(
            xt_hi[:, :, :, c * TLOC:(c + 1) * TLOC])
        im["xl8"] = np.ascontiguousarray(
            xt_lo[:, :, :, c * TLOC:(c + 1) * TLOC])
        in_maps.append(im)
    return in_maps


def kernel(x, gW1, gb1, gW2, gb2, W1, b1, W2, b2, _trace=False):
    from concourse.bass_utils import run_bass_kernel_spmd

    if "nc" not in _CACHE:
        _CACHE["nc"] = _build_nc()
    nc = _CACHE["nc"]
    args = [np.asarray(a, dtype=np.float32)
            for a in (x, gW1, gb1, gW2, gb2, W1, b1, W2, b2)]
    in_maps = _host_inputs(*args)
    try:
        kres = run_bass_kernel_spmd(nc, in_maps, core_ids=list(range(NCORES)),
                                    trace=_trace)
    except ModuleNotFoundError:
        kres = run_bass_kernel_spmd(nc, in_maps, core_ids=list(range(NCORES)),
                                    trace=False)
    _CACHE["last_result"] = kres
    # out_d [128, 4, ROWS]: out[p, oc, r] -> full[b = c*16+r, o = oc*128+p]
    outs = []
    for c in range(NCORES):
        arr = kres.results[c]["out"]                  # [128, 4, 16]
        outs.append(np.transpose(arr, (2, 1, 0)).reshape(ROWS, O))
    out = np.concatenate(outs, axis=0)
    return out.reshape(B, 1, 1, O).astype(np.float32)
